# revision 22
# baseline (speedup 1.0000x reference)
"""Causal multi-head attention kernel for Trainium2 (Bass/Tile), 8-core SPMD.

Problem: bs=32 (batch*heads), n=2048, hs=128, fp32, causal mask.
Sharding: bs axis split across 8 cores (4 heads per core), no communication.

Per-head algorithm (flash-style, no running max -- scores are ~N(0,1) so exp
is safe in fp32):
  S^T[k, q] = (K^T tile).T @ Q^T          (PE, bf16 in / fp32 PSUM out)
  P^T = exp(S^T / sqrt(dk))               (ACT, PSUM -> SBUF, bf16 out)
  zero strictly-upper triangle of each diagonal 128x128 block (GpSimd)
  [O | denom] accumulated over k-tiles:    (PE, bf16)
      out[q, 0:128+1] += (P^T tile).T @ [V | 1]
  O_norm = O * approx_recip(denom)        (DVE: recip_approx_fast + mul
                                           straight from PSUM)

DMA strategy: one full-head DMA per tensor (128 descriptors each) --
descriptor generation is the startup bottleneck, so kt/v ride the SP HWDGE
while qt rides the Activation HWDGE in parallel.  Output tiles are batched
8 q-tiles per store (2KB+ descriptors).

Layouts (prepared host-side): qt/kt [h, 128, 2048] bf16; vext
[h, 128, 16, 129] bf16 with the ones-column for the softmax denominator;
out [h, 128, 16, 128] f32 (partition-major, untransposed host-side after).
"""

import math
import os
from contextlib import ExitStack

import numpy as np

BS, N, HS = 32, 2048, 128
NCORES = 8
HEADS_PER_CORE = BS // NCORES
P = 128                      # partitions / head-dim / k-tile
QB = 512                     # q slot width in S^T super-tiles
NKT = N // P                 # 16 k-tiles per head
NQB = N // QB                # 4 q blocks per head
NQT = N // P                 # 16 q tiles per head
STB = 8                      # q-tiles batched per output store

# diag tile d = j % 4: computed q-range within its 512 slot is
# [128*d, 512) -- exactly the causally-needed columns.
def _diag_qs_w(d):
    return 128 * d, QB - 128 * d


SLOTS = 2                    # 512-col slots per S^T PSUM super-tile


def _sblocks():
    """S^T tiles grouped into <=SLOTS-tile PSUM super-tile chunks per j.

    Returns (chunks, off, col): chunks is a list of
    {tiles: [(j, b, qs, w, diag, local0)], act_lo, act_hi, pt_col};
    local0 is the tile's 512-aligned slot start inside the super-tile
    (diag tiles right-aligned so the exp region is contiguous).
    off[(j, b)] is the P^T slab column of that tile."""
    off = {}
    col = 0
    chunks = []
    for j in range(NKT):
        tiles = []
        for b in range(j // 4, NQB):
            if b == j // 4:
                dqs, w = _diag_qs_w(j % 4)
                tiles.append((j, b, QB * b + dqs, w, True))
            else:
                tiles.append((j, b, QB * b, QB, False))
        for c0 in range(0, len(tiles), SLOTS):
            group = tiles[c0 : c0 + SLOTS]
            gtiles = []
            local = 0
            act_lo = None
            pt_col = col
            for (tj, tb, qs, w, diag) in group:
                local0 = local + (QB - w)   # right-aligned in its 512 slot
                if act_lo is None:
                    act_lo = local0
                gtiles.append((tj, tb, qs, w, diag, local0))
                off[(tj, tb)] = col
                col += w
                local += QB
            chunks.append(
                dict(tiles=gtiles, act_lo=act_lo, act_hi=local, pt_col=pt_col)
            )
    return chunks, off, col


def build_bass():
    import concourse.mybir as mybir
    import concourse.tile as tile
    from concourse import bacc

    nc = bacc.Bacc("TRN2", target_bir_lowering=False, debug=False, num_devices=8)
    f32 = mybir.dt.float32
    bf16 = mybir.dt.bfloat16

    qt_d = nc.dram_tensor("qt", [HEADS_PER_CORE, P, N], bf16, kind="ExternalInput")
    kt_d = nc.dram_tensor("kt", [HEADS_PER_CORE, P, N], bf16, kind="ExternalInput")
    v_d = nc.dram_tensor(
        "vext", [HEADS_PER_CORE, P, NKT, HS + 1], bf16, kind="ExternalInput"
    )
    out_d = nc.dram_tensor(
        "out", [HEADS_PER_CORE, P, NQT, HS], f32, kind="ExternalOutput"
    )

    scale = 1.0 / math.sqrt(float(HS))
    chunks, pt_off, pt_cols = _sblocks()
    nchunks = len(chunks)
    # Last-head own-AV emission thresholds: AV tile t may only be emitted
    # once the S chunk holding exp(j=t) is >= 3 chunks old -- with s_psum
    # bufs=3 that exp is then structurally complete (the PE cannot run more
    # than 3 chunks ahead), so the in-order PE never blocks on a fresh exp.
    # chunk_of(j) = 2j+1 for j<8 (two chunks per j), j+8 for j>=8.
    PACE = int(os.environ.get("KERNEL_PACE", "20"))

    def _own_thresh(t):
        need_chunk = (2 * t + 1 if t < 8 else t + 8) + 3
        jdone = need_chunk // 2 if need_chunk < 16 else need_chunk - 8
        return jdone if jdone <= 15 else None

    with ExitStack() as ctx:
        tc = ctx.enter_context(tile.TileContext(nc))
        qt_pool = ctx.enter_context(tc.tile_pool(name="qt", bufs=3))
        kt_pool = ctx.enter_context(tc.tile_pool(name="kt", bufs=3))
        v_pool = ctx.enter_context(tc.tile_pool(name="vext", bufs=3))
        pt_pool = ctx.enter_context(tc.tile_pool(name="pt", bufs=2))
        o_pool = ctx.enter_context(tc.tile_pool(name="o", bufs=4))
        r_pool = ctx.enter_context(tc.tile_pool(name="recip", bufs=4))
        s_psum = ctx.enter_context(tc.tile_pool(name="spsum", bufs=3, space="PSUM"))
        o_psum = ctx.enter_context(tc.tile_pool(name="opsum", bufs=2, space="PSUM"))
        # s super-tiles are [128, 1024] = 2 banks x 3 bufs; o tiles 1 bank x 2

        def emit_loads(h):
            # kt/v on the SP HWDGE, qt on the Activation HWDGE: the two
            # descriptor generators run in parallel.  Descriptor generation
            # is serial per DGE at ~30ns/descriptor (128 descs per
            # full-partition tile), so head 0 -- the startup critical path --
            # loads kt/qt in column halves: the first halves' semaphores
            # release ~3.5us before the full tensors would, and the reordered
            # first chunks (see below) only touch the first halves.
            kt = kt_pool.tile([P, N], bf16, tag="kt", name=f"kt_{h}")
            qt = qt_pool.tile([P, N], bf16, tag="qt", name=f"qt_{h}")
            if h == 0:
                half = N // 2          # kt split: j<8 | j>=8
                qsp = 3 * QB           # qt split: b<=2 | b=3
                nc.sync.dma_start(kt[:, :half], kt_d.ap()[h][:, :half])
                nc.scalar.dma_start(qt[:, :qsp], qt_d.ap()[h][:, :qsp])
                nc.sync.dma_start(kt[:, half:], kt_d.ap()[h][:, half:])
                nc.scalar.dma_start(qt[:, qsp:], qt_d.ap()[h][:, qsp:])
            else:
                nc.sync.dma_start(kt[:], kt_d.ap()[h])
                nc.scalar.dma_start(qt[:], qt_d.ap()[h])
            v = v_pool.tile([P, NKT, HS + 1], bf16, tag="v", name=f"v_{h}")
            nc.sync.dma_start(v[:], v_d.ap()[h])
            return qt, kt, v

        def emit_s_chunk(ch, pt_t, qt, kt):
            s_t = s_psum.tile([P, SLOTS * QB], mybir.dt.float32)
            diag_zero = None
            for (j, b, qs, w, diag, l0) in ch["tiles"]:
                nc.tensor.matmul(
                    s_t[:, l0 : l0 + w],
                    kt[:, j * P : (j + 1) * P],
                    qt[:, qs : qs + w],
                    start=True,
                    stop=True,
                )
                if diag:
                    diag_zero = pt_off[(j, b)]
            lo, hi = ch["act_lo"], ch["act_hi"]
            nc.scalar.activation(
                pt_t[:, ch["pt_col"] : ch["pt_col"] + (hi - lo)],
                s_t[:, lo:hi],
                mybir.ActivationFunctionType.Exp,
                scale=scale,
            )
            if diag_zero is not None:
                # zero the strictly-upper triangle (k > q) of the exp'd
                # diagonal block in SBUF on the otherwise-idle GpSimd
                blk = pt_t[:, diag_zero : diag_zero + P]
                nc.gpsimd.affine_select(
                    out=blk,
                    in_=blk,
                    compare_op=mybir.AluOpType.is_ge,
                    fill=0.0,
                    base=0,
                    pattern=[[1, P]],
                    channel_multiplier=-1,
                )

        o2_cache = [None, 1]

        def emit_av_tile(h, t, pt_t, v, o_big):
            """AV + denom + normalize for one q-tile; store every STB tiles.

            Two [128,129] AV accumulators share one PSUM bank ([128,2,129]
            fits in a 2KB-per-partition bank), so 2 pool bufs give 4 slots
            and the AV stream never stalls on a bank being normalized."""
            b = t // 4
            if o2_cache[0] is None or o2_cache[1] == 1:
                o2_cache[0] = o_psum.tile([P, 2 * (HS + 1)], mybir.dt.float32, tag="o2", name=f"o2_{h}_{t}")
                o2_cache[1] = 0
            else:
                o2_cache[1] = 1
            sl = o2_cache[1] * (HS + 1)
            o_t = o2_cache[0][:, sl : sl + HS + 1]
            for j in range(t + 1):
                qs = QB * b + (128 * (j % 4) if b == j // 4 else 0)
                col = pt_off[(j, b)] + (P * t - qs)
                nc.tensor.matmul(
                    o_t,
                    pt_t[:, col : col + P],
                    v[:, j, :],
                    start=(j == 0),
                    stop=(j == t),
                )
            recip = r_pool.tile([P, 1], mybir.dt.float32, tag="recip")
            nc.vector.reciprocal_approx_fast(recip[:], o_t[:, HS : HS + 1])
            nc.vector.tensor_scalar_mul(
                o_big[:, t % STB, :], o_t[:, :HS], recip[:]
            )
            if t % STB == STB - 1:
                # two partition strips: halves the tail latency of the final
                # store (two DGE contexts generate descriptors in parallel)
                for p0 in (0, P // 2):
                    nc.sync.dma_start(
                        out_d.ap()[h][p0 : p0 + P // 2, t - (STB - 1) : t + 1],
                        o_big[p0 : p0 + P // 2],
                    )

        def get_obig(h, t, cache):
            if t % STB == 0:
                cache[0] = o_pool.tile(
                    [P, STB, HS], mybir.dt.float32, tag="obig", name=f"ob_{h}_{t}"
                )
            return cache[0]

        # Cross-head interleave: head h-1's AV q-tiles are spread between head
        # h's S chunks (their exp inputs are a full phase old, so the in-order
        # PE never blocks on them), front-loaded to finish by chunk 16 of 20
        # so the tail of each phase is pure S and ACT stays fed across the
        # head boundary.  The LAST head additionally drains its own AV with
        # LAG=2 behind its S chunks (s_psum bufs=2 forces the exp to be
        # complete by then), so the kernel doesn't end with a serial AV phase.
        # (Own-lag on EVERY head was measured slower: the trailing own-AV
        # tiles block the next head's S chunks in the in-order PE stream and
        # starve ACT ~3us at every head boundary.)
        av_prev = None
        ob_cache = [None]
        loaded = {0: emit_loads(0)}
        for h in range(HEADS_PER_CORE):
            if h + 1 < HEADS_PER_CORE:
                loaded[h + 1] = emit_loads(h + 1)
            qt, kt, v = loaded[h]
            pt_t = pt_pool.tile([P, pt_cols], bf16, tag="pt", name=f"pt_{h}")
            last = h == HEADS_PER_CORE - 1
            own_cache = [None]
            done_av = 0
            own_av = 0
            # Head 0: consume only first-half kt/qt columns (j<8, b<2) in the
            # first four chunks so the S pass starts before the second-half
            # loads land.  Chunks 0,2,4,6 are the diag-side chunks of j=0..3.
            order = ([0, 2, 4, 6, 8, 10, 12, 14, 1, 3, 5, 7, 9, 11, 13, 15]
                     + list(range(16, nchunks)) if h == 0 else range(nchunks))
            for i, ci in enumerate(order):
                ch = chunks[ci]
                emit_s_chunk(ch, pt_t, qt, kt)
                if av_prev is not None:
                    ph, ppt, pv = av_prev
                    while done_av < NQT and done_av * PACE < (i + 1) * NQT:
                        emit_av_tile(ph, done_av, ppt, pv,
                                     get_obig(ph, done_av, ob_cache))
                        done_av += 1
                if last:
                    jdone = ch["tiles"][-1][0]
                    while own_av < NQT and (
                        _own_thresh(own_av) is not None
                        and jdone >= _own_thresh(own_av)
                    ):
                        emit_av_tile(h, own_av, pt_t, v,
                                     get_obig(h, own_av, own_cache))
                        own_av += 1
            if av_prev is not None:
                ph, ppt, pv = av_prev
                while done_av < NQT:
                    emit_av_tile(ph, done_av, ppt, pv,
                                 get_obig(ph, done_av, ob_cache))
                    done_av += 1
            if last:
                while own_av < NQT:
                    emit_av_tile(h, own_av, pt_t, v,
                                 get_obig(h, own_av, own_cache))
                    own_av += 1
            av_prev = (h, pt_t, v)

    nc.compile()
    return nc


_NC_CACHE = None


def _get_nc():
    global _NC_CACHE
    if _NC_CACHE is None:
        _NC_CACHE = build_bass()
    return _NC_CACHE


def _is_causal_mask(mask: np.ndarray) -> bool:
    if mask.shape != (BS, N, N) or mask.dtype != np.bool_:
        return False
    tri = np.triu(np.ones((N, N), dtype=np.bool_), k=1)
    if not np.array_equal(mask[0], tri):
        return False
    # all batch entries identical
    return bool((mask == mask[0]).all())


def _numpy_fallback(QW, KW, VW, dk, mask):
    out = np.empty((BS, N, HS), dtype=np.float32)
    inv = 1.0 / np.sqrt(np.float32(dk))
    for i in range(BS):
        s = (QW[i] @ KW[i].T) * inv
        s = np.where(mask[i], -np.inf, s)
        s = s - s.max(axis=-1, keepdims=True)
        e = np.exp(s)
        out[i] = (e @ VW[i]) / e.sum(axis=-1, keepdims=True)
    return out


def _prepare_in_maps(QW, KW, VW):
    import ml_dtypes

    in_maps = []
    for c in range(NCORES):
        sl = slice(c * HEADS_PER_CORE, (c + 1) * HEADS_PER_CORE)
        qt = np.ascontiguousarray(
            QW[sl].transpose(0, 2, 1)).astype(ml_dtypes.bfloat16)
        kt = np.ascontiguousarray(
            KW[sl].transpose(0, 2, 1)).astype(ml_dtypes.bfloat16)
        # vext[h, p, j, c] = V[h, 128j+p, c], ones in column HS
        vext = np.empty((HEADS_PER_CORE, N, HS + 1), dtype=ml_dtypes.bfloat16)
        vext[:, :, :HS] = VW[sl].astype(ml_dtypes.bfloat16)
        vext[:, :, HS] = 1.0
        vext = np.ascontiguousarray(
            vext.reshape(HEADS_PER_CORE, NKT, P, HS + 1).transpose(0, 2, 1, 3)
        )
        in_maps.append({"qt": qt, "kt": kt, "vext": vext})
    return in_maps


def _run(QW, KW, VW, trace=False, **spmd_kwargs):
    from concourse import bass_utils

    nc = _get_nc()
    in_maps = _prepare_in_maps(QW, KW, VW)
    res = bass_utils.run_bass_kernel_spmd(
        nc, in_maps, core_ids=list(range(NCORES)), trace=trace, **spmd_kwargs
    )
    # out[h, p, t, c] -> O[h, 128t+p, c]
    out = np.concatenate(
        [r["out"].transpose(0, 2, 1, 3).reshape(HEADS_PER_CORE, N, HS)
         for r in res.results],
        axis=0,
    )
    return out, res


def kernel(QW, KW, VW, dk, mask):
    QW = np.asarray(QW, dtype=np.float32)
    KW = np.asarray(KW, dtype=np.float32)
    VW = np.asarray(VW, dtype=np.float32)
    mask = np.asarray(mask)
    if int(dk) != HS or not _is_causal_mask(mask):
        return _numpy_fallback(QW, KW, VW, int(dk), mask)
    out, _ = _run(QW, KW, VW, trace=bool(int(os.environ.get("KERNEL_TRACE", "0"))))
    return out


# revision 23
# speedup vs baseline: 1.0405x; 1.0405x over previous
"""Causal multi-head attention kernel for Trainium2 (Bass/Tile), 8-core SPMD.

Problem: bs=32 (batch*heads), n=2048, hs=128, fp32, causal mask.
Sharding: bs axis split across 8 cores (4 heads per core), no communication.

Per-head algorithm (flash-style, no running max -- scores are ~N(0,1) so exp
is safe in fp32):
  S^T[k, q] = (K^T tile).T @ Q^T          (PE, bf16 in / fp32 PSUM out)
  P^T = exp(S^T / sqrt(dk))               (ACT, PSUM -> SBUF, bf16 out)
  zero strictly-upper triangle of each diagonal 128x128 block (GpSimd)
  [O | denom] accumulated over k-tiles:    (PE, bf16)
      out[q, 0:128+1] += (P^T tile).T @ [V | 1]
  O_norm = O * approx_recip(denom)        (DVE: recip_approx_fast + mul
                                           straight from PSUM)

DMA strategy: one full-head DMA per tensor (128 descriptors each) --
descriptor generation is the startup bottleneck, so kt/v ride the SP HWDGE
while qt rides the Activation HWDGE in parallel.  Output tiles are batched
8 q-tiles per store (2KB+ descriptors).

Layouts (prepared host-side): qt/kt [h, 128, 2048] bf16; vext
[h, 128, 16, 129] bf16 with the ones-column for the softmax denominator;
out [h, 128, 16, 128] f32 (partition-major, untransposed host-side after).
"""

import math
import os
from contextlib import ExitStack

import numpy as np

BS, N, HS = 32, 2048, 128
NCORES = 8
HEADS_PER_CORE = BS // NCORES
P = 128                      # partitions / head-dim / k-tile
QB = 512                     # q slot width in S^T super-tiles
NKT = N // P                 # 16 k-tiles per head
NQB = N // QB                # 4 q blocks per head
NQT = N // P                 # 16 q tiles per head
STB = 8                      # q-tiles batched per output store

# diag tile d = j % 4: computed q-range within its 512 slot is
# [128*d, 512) -- exactly the causally-needed columns.
def _diag_qs_w(d):
    return 128 * d, QB - 128 * d


SLOTS = 2                    # 512-col slots per S^T PSUM super-tile


def _sblocks():
    """S^T tiles grouped into <=SLOTS-tile PSUM super-tile chunks per j.

    Returns (chunks, off, col): chunks is a list of
    {tiles: [(j, b, qs, w, diag, local0)], act_lo, act_hi, pt_col};
    local0 is the tile's 512-aligned slot start inside the super-tile
    (diag tiles right-aligned so the exp region is contiguous).
    off[(j, b)] is the P^T slab column of that tile."""
    off = {}
    col = 0
    chunks = []
    for j in range(NKT):
        tiles = []
        for b in range(j // 4, NQB):
            if b == j // 4:
                dqs, w = _diag_qs_w(j % 4)
                tiles.append((j, b, QB * b + dqs, w, True))
            else:
                tiles.append((j, b, QB * b, QB, False))
        for c0 in range(0, len(tiles), SLOTS):
            group = tiles[c0 : c0 + SLOTS]
            gtiles = []
            local = 0
            act_lo = None
            pt_col = col
            for (tj, tb, qs, w, diag) in group:
                local0 = local + (QB - w)   # right-aligned in its 512 slot
                if act_lo is None:
                    act_lo = local0
                gtiles.append((tj, tb, qs, w, diag, local0))
                off[(tj, tb)] = col
                col += w
                local += QB
            chunks.append(
                dict(tiles=gtiles, act_lo=act_lo, act_hi=local, pt_col=pt_col)
            )
    return chunks, off, col


def build_bass():
    import concourse.mybir as mybir
    import concourse.tile as tile
    from concourse import bacc

    nc = bacc.Bacc("TRN2", target_bir_lowering=False, debug=False, num_devices=8)
    f32 = mybir.dt.float32
    bf16 = mybir.dt.bfloat16

    qt_d = nc.dram_tensor("qt", [HEADS_PER_CORE, P, N], bf16, kind="ExternalInput")
    kt_d = nc.dram_tensor("kt", [HEADS_PER_CORE, P, N], bf16, kind="ExternalInput")
    v_d = nc.dram_tensor(
        "vext", [HEADS_PER_CORE, P, NKT, HS + 1], bf16, kind="ExternalInput"
    )
    out_d = nc.dram_tensor(
        "out", [HEADS_PER_CORE, P, NQT, HS], f32, kind="ExternalOutput"
    )

    scale = 1.0 / math.sqrt(float(HS))
    chunks, pt_off, pt_cols = _sblocks()
    nchunks = len(chunks)
    # Last-head own-AV emission thresholds: AV tile t may only be emitted
    # once the S chunk holding exp(j=t) is >= 3 chunks old -- with s_psum
    # bufs=3 that exp is then structurally complete (the PE cannot run more
    # than 3 chunks ahead), so the in-order PE never blocks on a fresh exp.
    # chunk_of(j) = 2j+1 for j<8 (two chunks per j), j+8 for j>=8.
    PACE = int(os.environ.get("KERNEL_PACE", "20"))

    def _own_thresh(t):
        need_chunk = (2 * t + 1 if t < 8 else t + 8) + 3
        jdone = need_chunk // 2 if need_chunk < 16 else need_chunk - 8
        return jdone if jdone <= 15 else None

    with ExitStack() as ctx:
        tc = ctx.enter_context(tile.TileContext(nc))
        qt_pool = ctx.enter_context(tc.tile_pool(name="qt", bufs=3))
        kt_pool = ctx.enter_context(tc.tile_pool(name="kt", bufs=3))
        v_pool = ctx.enter_context(tc.tile_pool(name="vext", bufs=3))
        pt_pool = ctx.enter_context(tc.tile_pool(name="pt", bufs=2))
        o_pool = ctx.enter_context(tc.tile_pool(name="o", bufs=4))
        r_pool = ctx.enter_context(tc.tile_pool(name="recip", bufs=4))
        s_psum = ctx.enter_context(tc.tile_pool(name="spsum", bufs=3, space="PSUM"))
        o_psum = ctx.enter_context(tc.tile_pool(name="opsum", bufs=2, space="PSUM"))
        # s super-tiles are [128, 1024] = 2 banks x 3 bufs; o tiles 1 bank x 2

        def emit_loads(h):
            # kt/v on the SP HWDGE, qt on the Activation HWDGE: the two
            # descriptor generators run in parallel.  Descriptor generation
            # is serial per DGE at ~30ns/descriptor (128 descs per
            # full-partition tile), so head 0 -- the startup critical path --
            # loads kt/qt in column halves: the first halves' semaphores
            # release ~3.5us before the full tensors would, and the reordered
            # first chunks (see below) only touch the first halves.
            kt = kt_pool.tile([P, N], bf16, tag="kt", name=f"kt_{h}")
            qt = qt_pool.tile([P, N], bf16, tag="qt", name=f"qt_{h}")
            if h == 0:
                half = N // 2          # kt split: j<8 | j>=8
                qsp = 3 * QB           # qt split: b<=2 | b=3
                nc.sync.dma_start(kt[:, :half], kt_d.ap()[h][:, :half])
                nc.scalar.dma_start(qt[:, :qsp], qt_d.ap()[h][:, :qsp])
                nc.sync.dma_start(kt[:, half:], kt_d.ap()[h][:, half:])
                nc.scalar.dma_start(qt[:, qsp:], qt_d.ap()[h][:, qsp:])
            else:
                nc.sync.dma_start(kt[:], kt_d.ap()[h])
                nc.scalar.dma_start(qt[:], qt_d.ap()[h])
            v = v_pool.tile([P, NKT, HS + 1], bf16, tag="v", name=f"v_{h}")
            nc.sync.dma_start(v[:], v_d.ap()[h])
            return qt, kt, v

        def emit_s_chunk(ch, pt_t, qt, kt):
            s_t = s_psum.tile([P, SLOTS * QB], mybir.dt.float32)
            diag_zero = None
            for (j, b, qs, w, diag, l0) in ch["tiles"]:
                nc.tensor.matmul(
                    s_t[:, l0 : l0 + w],
                    kt[:, j * P : (j + 1) * P],
                    qt[:, qs : qs + w],
                    start=True,
                    stop=True,
                )
                if diag:
                    diag_zero = pt_off[(j, b)]
            lo, hi = ch["act_lo"], ch["act_hi"]
            nc.scalar.activation(
                pt_t[:, ch["pt_col"] : ch["pt_col"] + (hi - lo)],
                s_t[:, lo:hi],
                mybir.ActivationFunctionType.Exp,
                scale=scale,
            )
            if diag_zero is not None:
                # zero the strictly-upper triangle (k > q) of the exp'd
                # diagonal block in SBUF on the otherwise-idle GpSimd
                blk = pt_t[:, diag_zero : diag_zero + P]
                nc.gpsimd.affine_select(
                    out=blk,
                    in_=blk,
                    compare_op=mybir.AluOpType.is_ge,
                    fill=0.0,
                    base=0,
                    pattern=[[1, P]],
                    channel_multiplier=-1,
                )

        def emit_av_tile(h, t, pt_t, v, o_big):
            """AV + denom + normalize for one q-tile; store every STB tiles."""
            b = t // 4
            o_t = o_psum.tile([P, HS + 1], mybir.dt.float32)
            for j in range(t + 1):
                qs = QB * b + (128 * (j % 4) if b == j // 4 else 0)
                col = pt_off[(j, b)] + (P * t - qs)
                nc.tensor.matmul(
                    o_t[:],
                    pt_t[:, col : col + P],
                    v[:, j, :],
                    start=(j == 0),
                    stop=(j == t),
                )
            recip = r_pool.tile([P, 1], mybir.dt.float32, tag="recip")
            nc.vector.reciprocal_approx_fast(recip[:], o_t[:, HS : HS + 1])
            nc.vector.tensor_scalar_mul(
                o_big[:, t % STB, :], o_t[:, :HS], recip[:]
            )
            if t % STB == STB - 1:
                # two partition strips: halves the tail latency of the final
                # store (two DGE contexts generate descriptors in parallel)
                for p0 in (0, P // 2):
                    nc.sync.dma_start(
                        out_d.ap()[h][p0 : p0 + P // 2, t - (STB - 1) : t + 1],
                        o_big[p0 : p0 + P // 2],
                    )

        def get_obig(h, t, cache):
            if t % STB == 0:
                cache[0] = o_pool.tile(
                    [P, STB, HS], mybir.dt.float32, tag="obig", name=f"ob_{h}_{t}"
                )
            return cache[0]

        # Cross-head interleave: head h-1's AV q-tiles are spread between head
        # h's S chunks (their exp inputs are a full phase old, so the in-order
        # PE never blocks on them), front-loaded to finish by chunk 16 of 20
        # so the tail of each phase is pure S and ACT stays fed across the
        # head boundary.  The LAST head additionally drains its own AV with
        # LAG=2 behind its S chunks (s_psum bufs=2 forces the exp to be
        # complete by then), so the kernel doesn't end with a serial AV phase.
        # (Own-lag on EVERY head was measured slower: the trailing own-AV
        # tiles block the next head's S chunks in the in-order PE stream and
        # starve ACT ~3us at every head boundary.)
        av_prev = None
        ob_cache = [None]
        loaded = {0: emit_loads(0)}
        for h in range(HEADS_PER_CORE):
            if h + 1 < HEADS_PER_CORE:
                loaded[h + 1] = emit_loads(h + 1)
            qt, kt, v = loaded[h]
            pt_t = pt_pool.tile([P, pt_cols], bf16, tag="pt", name=f"pt_{h}")
            last = h == HEADS_PER_CORE - 1
            own_cache = [None]
            done_av = 0
            own_av = 0
            # Head 0: consume only first-half kt/qt columns (j<8, b<2) in the
            # first four chunks so the S pass starts before the second-half
            # loads land.  Chunks 0,2,4,6 are the diag-side chunks of j=0..3.
            order = ([0, 2, 4, 6, 8, 10, 12, 14, 1, 3, 5, 7, 9, 11, 13, 15]
                     + list(range(16, nchunks)) if h == 0 else range(nchunks))
            for i, ci in enumerate(order):
                ch = chunks[ci]
                emit_s_chunk(ch, pt_t, qt, kt)
                if av_prev is not None:
                    ph, ppt, pv = av_prev
                    while done_av < NQT and done_av * PACE < (i + 1) * NQT:
                        emit_av_tile(ph, done_av, ppt, pv,
                                     get_obig(ph, done_av, ob_cache))
                        done_av += 1
                if last:
                    jdone = ch["tiles"][-1][0]
                    while own_av < NQT and (
                        _own_thresh(own_av) is not None
                        and jdone >= _own_thresh(own_av)
                    ):
                        emit_av_tile(h, own_av, pt_t, v,
                                     get_obig(h, own_av, own_cache))
                        own_av += 1
            if av_prev is not None:
                ph, ppt, pv = av_prev
                while done_av < NQT:
                    emit_av_tile(ph, done_av, ppt, pv,
                                 get_obig(ph, done_av, ob_cache))
                    done_av += 1
            if last:
                while own_av < NQT:
                    emit_av_tile(h, own_av, pt_t, v,
                                 get_obig(h, own_av, own_cache))
                    own_av += 1
            av_prev = (h, pt_t, v)

    nc.compile()
    return nc


_NC_CACHE = None


def _get_nc():
    global _NC_CACHE
    if _NC_CACHE is None:
        _NC_CACHE = build_bass()
    return _NC_CACHE


def _is_causal_mask(mask: np.ndarray) -> bool:
    if mask.shape != (BS, N, N) or mask.dtype != np.bool_:
        return False
    tri = np.triu(np.ones((N, N), dtype=np.bool_), k=1)
    if not np.array_equal(mask[0], tri):
        return False
    # all batch entries identical
    return bool((mask == mask[0]).all())


def _numpy_fallback(QW, KW, VW, dk, mask):
    out = np.empty((BS, N, HS), dtype=np.float32)
    inv = 1.0 / np.sqrt(np.float32(dk))
    for i in range(BS):
        s = (QW[i] @ KW[i].T) * inv
        s = np.where(mask[i], -np.inf, s)
        s = s - s.max(axis=-1, keepdims=True)
        e = np.exp(s)
        out[i] = (e @ VW[i]) / e.sum(axis=-1, keepdims=True)
    return out


def _prepare_in_maps(QW, KW, VW):
    import ml_dtypes

    in_maps = []
    for c in range(NCORES):
        sl = slice(c * HEADS_PER_CORE, (c + 1) * HEADS_PER_CORE)
        qt = np.ascontiguousarray(
            QW[sl].transpose(0, 2, 1)).astype(ml_dtypes.bfloat16)
        kt = np.ascontiguousarray(
            KW[sl].transpose(0, 2, 1)).astype(ml_dtypes.bfloat16)
        # vext[h, p, j, c] = V[h, 128j+p, c], ones in column HS
        vext = np.empty((HEADS_PER_CORE, N, HS + 1), dtype=ml_dtypes.bfloat16)
        vext[:, :, :HS] = VW[sl].astype(ml_dtypes.bfloat16)
        vext[:, :, HS] = 1.0
        vext = np.ascontiguousarray(
            vext.reshape(HEADS_PER_CORE, NKT, P, HS + 1).transpose(0, 2, 1, 3)
        )
        in_maps.append({"qt": qt, "kt": kt, "vext": vext})
    return in_maps


def _run(QW, KW, VW, trace=False, **spmd_kwargs):
    from concourse import bass_utils

    nc = _get_nc()
    in_maps = _prepare_in_maps(QW, KW, VW)
    res = bass_utils.run_bass_kernel_spmd(
        nc, in_maps, core_ids=list(range(NCORES)), trace=trace, **spmd_kwargs
    )
    # out[h, p, t, c] -> O[h, 128t+p, c]
    out = np.concatenate(
        [r["out"].transpose(0, 2, 1, 3).reshape(HEADS_PER_CORE, N, HS)
         for r in res.results],
        axis=0,
    )
    return out, res


def kernel(QW, KW, VW, dk, mask):
    QW = np.asarray(QW, dtype=np.float32)
    KW = np.asarray(KW, dtype=np.float32)
    VW = np.asarray(VW, dtype=np.float32)
    mask = np.asarray(mask)
    if int(dk) != HS or not _is_causal_mask(mask):
        return _numpy_fallback(QW, KW, VW, int(dk), mask)
    out, _ = _run(QW, KW, VW, trace=bool(int(os.environ.get("KERNEL_TRACE", "0"))))
    return out
